# revision 18
# baseline (speedup 1.0000x reference)
# Bahdanau-attention kernel for TRN2, data-parallel over batch across 8 NeuronCores.
#
# reference math (B=16, S=2048, H=1024):
#   h_proj = hidden @ W[:, :H].T                      [B, H]
#   e_proj = einsum('bsh,gh->bsg', enc, W[:, H:])     [B, S, H]
#   scores = tanh(h_proj[:,None,:] + e_proj + b)      [B, S, H]
#   logits = scores @ v                               [B, S]
#   out    = softmax(logits, -1)[:, None, :]          [B, 1, S]
#
# Per-core layout (2 batches/core):
#   All matmuls contract over h (or g), so every SBUF operand is laid out with
#   the contraction dim on partitions. Host pre-transposes/pre-tiles:
#     encT[bb, p, k, s] = enc[2i+bb, s, 128k+p]       (fp8 e4m3)
#     we[p, j, k, m] = 32 * W[128j+m, 1024+128k+p]    (fp8 e4m3, W_e^T pre-scaled)
#     wh[p, j, k, m] = W[128j+m, 128k+p]              (bf16, W_h^T for h_proj)
#     hiddenT[p, k, bb] = hidden[2i+bb, 128k+p]       (bf16)
#     bvec[p, j] = b[128j+p] (f32);  vvec[p, j] = v[128j+p] (bf16)
#   On chip per (batch, s-block of 512):
#     for each g-tile j: psum[g,s] = sum_kp DoubleRow-MM(we pair, encT pair)
#       (fp8 DoubleRow: one MM contracts two k-tiles -> 4 MMs per group, ~2x rate)
#     scoresT = tanh(psum/32 + (h_projT + b)[g])   (ACT, per-partition bias, bf16 out)
#     logits_psum[1,s] += v_tile(j).T @ scoresT    (PE, bf16)
#     then per-block EXP from PSUM (no max-subtraction; logits bounded by ||v||_1),
#     accumulated partials, one reciprocal, scaled output row.

import numpy as np
import ml_dtypes

import concourse.bass as bass
import concourse.mybir as mybir
import concourse.tile as tile
from concourse import bacc
from concourse.bass_utils import run_bass_kernel_spmd
from concourse.tile_rust import add_dep_helper

B, S, H = 16, 2048, 1024
NCORES = 8
BPC = B // NCORES          # batches per core
KT = H // 128              # contraction tiles
GT = H // 128              # output (g) tiles
SBLK = 512                 # s-block (one PSUM bank of f32)
NSB = S // SBLK

BF16 = mybir.dt.bfloat16
F32 = mybir.dt.float32
FP8 = mybir.dt.float8e4
WSCALE = 32.0              # W_e pre-scaled into fp8's sweet range; undone in tanh's scale

_CACHE = {}


def _build():
    nc = bacc.Bacc("TRN2", target_bir_lowering=False, debug=False, num_devices=NCORES)

    encT_d = nc.dram_tensor("encT", [BPC, 128, KT, S], FP8, kind="ExternalInput")
    wh_d = nc.dram_tensor("wh", [128, GT, KT, 128], BF16, kind="ExternalInput")
    we_d = nc.dram_tensor("we", [128, GT, KT, 128], FP8, kind="ExternalInput")
    hiddenT_d = nc.dram_tensor("hiddenT", [128, KT, BPC], BF16, kind="ExternalInput")
    bvec_d = nc.dram_tensor("bvec", [128, GT], F32, kind="ExternalInput")
    vvec_d = nc.dram_tensor("vvec", [128, GT], BF16, kind="ExternalInput")
    out_d = nc.dram_tensor("out", [BPC, S], F32, kind="ExternalOutput")

    ACT = mybir.ActivationFunctionType

    with tile.TileContext(nc) as tc:
        with (
            tc.tile_pool(name="const", bufs=1) as constp,
            tc.tile_pool(name="wp", bufs=1) as wp,
            tc.tile_pool(name="encp", bufs=1) as encp,
            tc.tile_pool(name="scp", bufs=2) as scp,
            tc.tile_pool(name="smallp", bufs=2) as smallp,
            tc.tile_pool(name="mps", bufs=3, space="PSUM") as mps,
            tc.tile_pool(name="lps", bufs=2, space="PSUM") as lps,
            tc.tile_pool(name="hps", bufs=2, space="PSUM") as hps,
        ):
            # --- tiny constants ---
            hiddenT_sb = constp.tile([128, KT, BPC], BF16, tag="hiddenT")
            nc.scalar.dma_start(out=hiddenT_sb[:], in_=hiddenT_d[:])
            b_sb = constp.tile([128, GT], F32, tag="bvec")
            nc.scalar.dma_start(out=b_sb[:], in_=bvec_d[:])
            v_sb = constp.tile([128, GT], BF16, tag="vvec")
            nc.scalar.dma_start(out=v_sb[:], in_=vvec_d[:])

            # --- weights: W_e^T in fp8 (DoubleRow main GEMM), W_h^T in bf16 ---
            we_sb = [None] * GT
            wh_sb = [None] * GT

            def load_we(j):
                t = wp.tile([128, KT, 128], FP8, name=f"we{j}", tag=f"we{j}")
                inst = nc.sync.dma_start(out=t[:], in_=we_d[:, j])
                we_sb[j] = t
                return inst

            def load_wh(j):
                t = wp.tile([128, KT, 128], BF16, name=f"wh{j}", tag=f"wh{j}")
                inst = nc.sync.dma_start(out=t[:], in_=wh_d[:, j])
                wh_sb[j] = t
                return inst

            # h_projT[j] = sum_k W_h(k,j).T @ hiddenT(k)  -> [128, BPC] per g-tile j,
            # then hb[:, j, bb] = h_projT + b (per-partition bias for the tanh).
            hb_sb = constp.tile([128, GT, BPC], F32, tag="hb")

            def hproj(j, collect=None):
                hp = hps.tile([128, BPC], F32, tag="hp")
                for k in range(KT):
                    mm = nc.tensor.matmul(
                        hp[:],
                        wh_sb[j][:, k, :],
                        hiddenT_sb[:, k, :],
                        start=(k == 0),
                        stop=(k == KT - 1),
                    )
                    if collect is not None:
                        collect.append(mm)
                nc.vector.tensor_scalar_add(hb_sb[:, j, :], hp[:], b_sb[:, j : j + 1])

            # DMA order tuned so the first main matmul group can start early:
            # We(j=0), Wh(j=0), enc b0 (first s-half), then alternating We/Wh columns.
            nc.scalar.dma_start(out=(t := wp.tile([128, KT, 128], FP8, name="we0", tag="we0"))[:], in_=we_d[:, 0])
            we_sb[0] = t
            nc.scalar.dma_start(out=(t2 := wp.tile([128, KT, 128], BF16, name="wh0", tag="wh0"))[:], in_=wh_d[:, 0])
            wh_sb[0] = t2
            hproj0_insts = []
            hproj(0, collect=hproj0_insts)

            enc_sb = [
                encp.tile([128, KT, S], FP8, name=f"enc{bb}", tag=f"enc{bb}")
                for bb in range(BPC)
            ]

            def load_enc(bb, lo, size):
                sl = slice(lo, lo + size)
                return nc.sync.dma_start(
                    out=enc_sb[bb][:, :, sl], in_=encT_d[bb][:, :, sl]
                )

            # DMA waves: what the first matmul group needs streams immediately;
            # later waves are gated on compute milestones (add_dep_helper below)
            # so the critical wave gets the full HBM bandwidth.
            dma_waves = [[], [], []]
            nc.scalar.dma_start(
                out=enc_sb[0][:, :, 0:SBLK], in_=encT_d[0][:, :, 0:SBLK]
            )
            for j in range(1, GT):
                dma_waves[0].append(load_we(j))
                dma_waves[0].append(load_wh(j))
                hproj(j)
            dma_waves[0].append(load_enc(0, SBLK, SBLK))
            dma_waves[1].append(load_enc(0, 2 * SBLK, 2 * SBLK))
            dma_waves[1].append(load_enc(1, 0, 2 * SBLK))
            dma_waves[2].append(load_enc(1, 2 * SBLK, 2 * SBLK))

            # --- main loop ---
            # Softmax note: no max-subtraction — |logits| <= ||v||_1 * max|tanh| ~ 9.6,
            # so exp() cannot overflow in f32 and the softmax ratio is unchanged.
            gates = {}
            mm_gate = None
            for bb in range(BPC):
                exps = smallp.tile([1, S], F32, name=f"exps{bb}", tag=f"exps{bb}")
                parts = smallp.tile([1, NSB], F32, name=f"parts{bb}", tag=f"parts{bb}")
                for sb in range(NSB):
                    sl = slice(sb * SBLK, (sb + 1) * SBLK)
                    lp = lps.tile([1, SBLK], F32, tag="lp")
                    for j in range(GT):
                        mp = mps.tile([128, SBLK], F32, tag="mp")
                        for kp in range(KT // 2):
                            mm = nc.tensor.matmul(
                                mp[:],
                                we_sb[j][:, 2 * kp : 2 * kp + 2, :],
                                enc_sb[bb][:, 2 * kp : 2 * kp + 2, sl],
                                start=(kp == 0),
                                stop=(kp == KT // 2 - 1),
                                perf_mode=mybir.MatmulPerfMode.DoubleRow,
                            )
                            if mm_gate is None:
                                mm_gate = mm
                        sc = scp.tile([128, SBLK], BF16, name=f"sc{j}", tag=f"sc{j}")
                        act_inst = nc.scalar.activation(
                            sc[:], mp[:], ACT.Tanh, bias=hb_sb[:, j, bb : bb + 1],
                            scale=1.0 / WSCALE,
                        )
                        gates[(bb, sb, j)] = act_inst
                        nc.tensor.matmul(
                            lp[:],
                            v_sb[:, j : j + 1],
                            sc[:],
                            start=(j == 0),
                            stop=(j == GT - 1),
                        )
                    nc.scalar.activation(
                        exps[:, sl], lp[:], ACT.Exp,
                        accum_out=parts[:, sb : sb + 1],
                    )

                ssum = smallp.tile([1, 1], F32, tag="ssum")
                nc.vector.tensor_reduce(
                    ssum[:], parts[:], axis=mybir.AxisListType.X,
                    op=mybir.AluOpType.add,
                )
                rsum = smallp.tile([1, 1], F32, tag="rsum")
                nc.vector.reciprocal(rsum[:], ssum[:])
                outrow = smallp.tile([1, S], F32, name=f"outrow{bb}", tag=f"outrow{bb}")
                nc.vector.tensor_scalar_mul(outrow[:], exps[:], rsum[:])
                nc.sync.dma_start(out=out_d[bb : bb + 1, :], in_=outrow[:])

            # gate the later DMA waves on compute progress: wave 1 after the
            # first tanh of (b0, sb0); wave 2 after (b0, sb1) finishes.
            wave_gates = [hproj0_insts[-1], mm_gate, gates[(0, 1, 7)]]
            for wv, gate in zip(dma_waves, wave_gates):
                for inst in wv:
                    add_dep_helper(
                        inst.ins, gate.ins, sync=True, reason="dma wave gating"
                    )

    nc.compile()
    return nc


def _get_nc():
    if "nc" not in _CACHE:
        _CACHE["nc"] = _build()
    return _CACHE["nc"]


def _make_in_maps(hidden, encoder_outputs, W, b, v):
    bf = ml_dtypes.bfloat16
    fp8 = ml_dtypes.float8_e4m3
    WT = np.ascontiguousarray(W.T)  # [2H, H]; WT[hin, gout]
    w_tiles = WT.reshape(2, KT, 128, GT, 128).transpose(0, 2, 3, 1, 4)  # [half, p, j, k, m]
    wh_host = np.ascontiguousarray(w_tiles[0]).astype(bf)
    we_host = np.ascontiguousarray(w_tiles[1] * WSCALE).astype(fp8)
    b_host = np.ascontiguousarray(b.reshape(GT, 128).T).astype(np.float32)
    v_host = np.ascontiguousarray(v.reshape(GT, 128).T).astype(bf)

    in_maps = []
    for i in range(NCORES):
        hs = hidden[BPC * i : BPC * (i + 1)]  # [BPC, H]
        es = encoder_outputs[BPC * i : BPC * (i + 1)]  # [BPC, S, H]
        hT = np.ascontiguousarray(
            hs.T.reshape(KT, 128, BPC).transpose(1, 0, 2)
        ).astype(bf)
        # [bb, p, k, s]: partition dim outermost so one DMA fills all k-tiles
        # of a column range with matching AP iteration order
        eT = np.ascontiguousarray(
            es.transpose(0, 2, 1).reshape(BPC, KT, 128, S).transpose(0, 2, 1, 3)
        ).astype(fp8)
        in_maps.append(
            {
                "encT": eT,
                "wh": wh_host,
                "we": we_host,
                "hiddenT": hT,
                "bvec": b_host,
                "vvec": v_host,
            }
        )
    return in_maps


def _run(in_maps, **kwargs):
    nc = _get_nc()
    try:
        return run_bass_kernel_spmd(
            nc, in_maps, core_ids=list(range(NCORES)), **kwargs
        )
    except Exception:
        # A first execution right after NEFF load has been seen to wedge the
        # device once; it recovers after a short pause. Retry once.
        import time as _time

        _time.sleep(20)
        return run_bass_kernel_spmd(
            nc, in_maps, core_ids=list(range(NCORES)), **kwargs
        )


def kernel(hidden, encoder_outputs, W, b, v):
    hidden = np.asarray(hidden, dtype=np.float32)
    encoder_outputs = np.asarray(encoder_outputs, dtype=np.float32)
    W = np.asarray(W, dtype=np.float32)
    b = np.asarray(b, dtype=np.float32)
    v = np.asarray(v, dtype=np.float32)

    in_maps = _make_in_maps(hidden, encoder_outputs, W, b, v)
    res = _run(in_maps)
    outs = [np.asarray(res.results[i]["out"], dtype=np.float32) for i in range(NCORES)]
    return np.concatenate(outs, axis=0).reshape(B, 1, S)


# revision 19
# speedup vs baseline: 1.0041x; 1.0041x over previous
# Bahdanau-attention kernel for TRN2, data-parallel over batch across 8 NeuronCores.
#
# reference math (B=16, S=2048, H=1024):
#   h_proj = hidden @ W[:, :H].T                      [B, H]
#   e_proj = einsum('bsh,gh->bsg', enc, W[:, H:])     [B, S, H]
#   scores = tanh(h_proj[:,None,:] + e_proj + b)      [B, S, H]
#   logits = scores @ v                               [B, S]
#   out    = softmax(logits, -1)[:, None, :]          [B, 1, S]
#
# Per-core layout (2 batches/core):
#   All matmuls contract over h (or g), so every SBUF operand is laid out with
#   the contraction dim on partitions. Host pre-transposes/pre-tiles:
#     encT[bb, p, k, s] = enc[2i+bb, s, 128k+p]       (fp8 e4m3)
#     we[p, j, k, m] = 32 * W[128j+m, 1024+128k+p]    (fp8 e4m3, W_e^T pre-scaled)
#     wh[p, j, k, m] = W[128j+m, 128k+p]              (bf16, W_h^T for h_proj)
#     hiddenT[p, k, bb] = hidden[2i+bb, 128k+p]       (bf16)
#     bvec[p, j] = b[128j+p] (f32);  vvec[p, j] = v[128j+p] (bf16)
#   On chip per (batch, s-block of 512):
#     for each g-tile j: psum[g,s] = sum_kp DoubleRow-MM(we pair, encT pair)
#       (fp8 DoubleRow: one MM contracts two k-tiles -> 4 MMs per group, ~2x rate)
#     scoresT = tanh(psum/32 + (h_projT + b)[g])   (ACT, per-partition bias, bf16 out)
#     logits_psum[1,s] += v_tile(j).T @ scoresT    (PE, bf16)
#     then per-block EXP from PSUM (no max-subtraction; logits bounded by ||v||_1),
#     accumulated partials, one reciprocal, scaled output row.

import numpy as np
import ml_dtypes

import concourse.bass as bass
import concourse.mybir as mybir
import concourse.tile as tile
from concourse import bacc
from concourse.bass_utils import run_bass_kernel_spmd
from concourse.tile_rust import add_dep_helper

B, S, H = 16, 2048, 1024
NCORES = 8
BPC = B // NCORES          # batches per core
KT = H // 128              # contraction tiles
GT = H // 128              # output (g) tiles
SBLK = 512                 # s-block (one PSUM bank of f32)
NSB = S // SBLK

BF16 = mybir.dt.bfloat16
F32 = mybir.dt.float32
FP8 = mybir.dt.float8e4
WSCALE = 32.0              # W_e pre-scaled into fp8's sweet range; undone in tanh's scale

_CACHE = {}


def _build():
    nc = bacc.Bacc("TRN2", target_bir_lowering=False, debug=False, num_devices=NCORES)

    encT_d = nc.dram_tensor("encT", [BPC, 128, KT, S], FP8, kind="ExternalInput")
    wh_d = nc.dram_tensor("wh", [128, GT, KT, 128], BF16, kind="ExternalInput")
    we_d = nc.dram_tensor("we", [128, GT, KT, 128], FP8, kind="ExternalInput")
    hiddenT_d = nc.dram_tensor("hiddenT", [128, KT, BPC], BF16, kind="ExternalInput")
    bvec_d = nc.dram_tensor("bvec", [128, GT], F32, kind="ExternalInput")
    vvec_d = nc.dram_tensor("vvec", [128, GT], BF16, kind="ExternalInput")
    out_d = nc.dram_tensor("out", [BPC, S], F32, kind="ExternalOutput")

    ACT = mybir.ActivationFunctionType

    with tile.TileContext(nc) as tc:
        with (
            tc.tile_pool(name="const", bufs=1) as constp,
            tc.tile_pool(name="wp", bufs=1) as wp,
            tc.tile_pool(name="encp", bufs=1) as encp,
            tc.tile_pool(name="scp", bufs=2) as scp,
            tc.tile_pool(name="smallp", bufs=2) as smallp,
            tc.tile_pool(name="mps", bufs=3, space="PSUM") as mps,
            tc.tile_pool(name="lps", bufs=2, space="PSUM") as lps,
            tc.tile_pool(name="hps", bufs=2, space="PSUM") as hps,
        ):
            # --- tiny constants ---
            hiddenT_sb = constp.tile([128, KT, BPC], BF16, tag="hiddenT")
            nc.sync.dma_start(out=hiddenT_sb[:], in_=hiddenT_d[:])
            b_sb = constp.tile([128, GT], F32, tag="bvec")
            nc.sync.dma_start(out=b_sb[:], in_=bvec_d[:])
            v_sb = constp.tile([128, GT], BF16, tag="vvec")
            nc.sync.dma_start(out=v_sb[:], in_=vvec_d[:])

            # --- weights: W_e^T in fp8 (DoubleRow main GEMM), W_h^T in bf16 ---
            we_sb = [None] * GT
            wh_sb = [None] * GT

            def load_we(j):
                t = wp.tile([128, KT, 128], FP8, name=f"we{j}", tag=f"we{j}")
                inst = nc.sync.dma_start(out=t[:], in_=we_d[:, j])
                we_sb[j] = t
                return inst

            def load_wh(j):
                t = wp.tile([128, KT, 128], BF16, name=f"wh{j}", tag=f"wh{j}")
                inst = nc.sync.dma_start(out=t[:], in_=wh_d[:, j])
                wh_sb[j] = t
                return inst

            # h_projT[j] = sum_k W_h(k,j).T @ hiddenT(k)  -> [128, BPC] per g-tile j,
            # then hb[:, j, bb] = h_projT + b (per-partition bias for the tanh).
            hb_sb = constp.tile([128, GT, BPC], F32, tag="hb")

            def hproj(j):
                hp = hps.tile([128, BPC], F32, tag="hp")
                for k in range(KT):
                    nc.tensor.matmul(
                        hp[:],
                        wh_sb[j][:, k, :],
                        hiddenT_sb[:, k, :],
                        start=(k == 0),
                        stop=(k == KT - 1),
                    )
                nc.vector.tensor_scalar_add(hb_sb[:, j, :], hp[:], b_sb[:, j : j + 1])

            # DMA order tuned so the first main matmul group can start early:
            # We(j=0), Wh(j=0), enc b0 (first s-half), then alternating We/Wh columns.
            load_we(0)
            load_wh(0)
            hproj(0)

            enc_sb = [
                encp.tile([128, KT, S], FP8, name=f"enc{bb}", tag=f"enc{bb}")
                for bb in range(BPC)
            ]

            def load_enc(bb, lo, size):
                sl = slice(lo, lo + size)
                return nc.sync.dma_start(
                    out=enc_sb[bb][:, :, sl], in_=encT_d[bb][:, :, sl]
                )

            # DMA waves: what the first matmul group needs streams immediately;
            # later waves are gated on compute milestones (add_dep_helper below)
            # so the critical wave gets the full HBM bandwidth.
            dma_waves = [[], [], []]
            load_enc(0, 0, SBLK)
            for j in range(1, GT):
                dma_waves[0].append(load_we(j))
                dma_waves[0].append(load_wh(j))
                hproj(j)
            dma_waves[0].append(load_enc(0, SBLK, SBLK))
            dma_waves[1].append(load_enc(0, 2 * SBLK, 2 * SBLK))
            dma_waves[1].append(load_enc(1, 0, 2 * SBLK))
            dma_waves[2].append(load_enc(1, 2 * SBLK, 2 * SBLK))

            # --- main loop ---
            # Softmax note: no max-subtraction — |logits| <= ||v||_1 * max|tanh| ~ 9.6,
            # so exp() cannot overflow in f32 and the softmax ratio is unchanged.
            gates = {}
            mm_gate = None
            for bb in range(BPC):
                exps = smallp.tile([1, S], F32, name=f"exps{bb}", tag=f"exps{bb}")
                parts = smallp.tile([1, NSB], F32, name=f"parts{bb}", tag=f"parts{bb}")
                for sb in range(NSB):
                    sl = slice(sb * SBLK, (sb + 1) * SBLK)
                    scs = []
                    for j in range(GT):
                        mp = mps.tile([128, SBLK], F32, tag="mp")
                        for kp in range(KT // 2):
                            mm = nc.tensor.matmul(
                                mp[:],
                                we_sb[j][:, 2 * kp : 2 * kp + 2, :],
                                enc_sb[bb][:, 2 * kp : 2 * kp + 2, sl],
                                start=(kp == 0),
                                stop=(kp == KT // 2 - 1),
                                perf_mode=mybir.MatmulPerfMode.DoubleRow,
                            )
                            if mm_gate is None:
                                mm_gate = mm
                        sc = scp.tile([128, SBLK], BF16, name=f"sc{j}", tag=f"sc{j}")
                        act_inst = nc.scalar.activation(
                            sc[:], mp[:], ACT.Tanh, bias=hb_sb[:, j, bb : bb + 1],
                            scale=1.0 / WSCALE,
                        )
                        gates[(bb, sb, j)] = act_inst
                        scs.append(sc)
                    lp = lps.tile([1, SBLK], F32, tag="lp")
                    for j in range(GT):
                        nc.tensor.matmul(
                            lp[:],
                            v_sb[:, j : j + 1],
                            scs[j][:],
                            start=(j == 0),
                            stop=(j == GT - 1),
                        )
                    nc.scalar.activation(
                        exps[:, sl], lp[:], ACT.Exp,
                        accum_out=parts[:, sb : sb + 1],
                    )

                ssum = smallp.tile([1, 1], F32, tag="ssum")
                nc.vector.tensor_reduce(
                    ssum[:], parts[:], axis=mybir.AxisListType.X,
                    op=mybir.AluOpType.add,
                )
                rsum = smallp.tile([1, 1], F32, tag="rsum")
                nc.vector.reciprocal(rsum[:], ssum[:])
                outrow = smallp.tile([1, S], F32, name=f"outrow{bb}", tag=f"outrow{bb}")
                nc.vector.tensor_scalar_mul(outrow[:], exps[:], rsum[:])
                nc.sync.dma_start(out=out_d[bb : bb + 1, :], in_=outrow[:])

            # gate the later DMA waves on compute progress: wave 1 after the
            # first tanh of (b0, sb0); wave 2 after (b0, sb1) finishes.
            for inst in dma_waves[0]:
                add_dep_helper(
                    inst.ins, mm_gate.ins, sync=True,
                    reason="dma wave 0 gated on first main matmul",
                )
            for inst in dma_waves[1]:
                add_dep_helper(
                    inst.ins, gates[(0, 0, 0)].ins, sync=True,
                    reason="dma wave 1 gated on first tanh",
                )
            for inst in dma_waves[2]:
                add_dep_helper(
                    inst.ins, gates[(0, 1, 7)].ins, sync=True,
                    reason="dma wave 2 gated on (b0,sb1) tanh",
                )

    nc.compile()
    return nc


def _get_nc():
    if "nc" not in _CACHE:
        _CACHE["nc"] = _build()
    return _CACHE["nc"]


def _make_in_maps(hidden, encoder_outputs, W, b, v):
    bf = ml_dtypes.bfloat16
    fp8 = ml_dtypes.float8_e4m3
    WT = np.ascontiguousarray(W.T)  # [2H, H]; WT[hin, gout]
    w_tiles = WT.reshape(2, KT, 128, GT, 128).transpose(0, 2, 3, 1, 4)  # [half, p, j, k, m]
    wh_host = np.ascontiguousarray(w_tiles[0]).astype(bf)
    we_host = np.ascontiguousarray(w_tiles[1] * WSCALE).astype(fp8)
    b_host = np.ascontiguousarray(b.reshape(GT, 128).T).astype(np.float32)
    v_host = np.ascontiguousarray(v.reshape(GT, 128).T).astype(bf)

    in_maps = []
    for i in range(NCORES):
        hs = hidden[BPC * i : BPC * (i + 1)]  # [BPC, H]
        es = encoder_outputs[BPC * i : BPC * (i + 1)]  # [BPC, S, H]
        hT = np.ascontiguousarray(
            hs.T.reshape(KT, 128, BPC).transpose(1, 0, 2)
        ).astype(bf)
        # [bb, p, k, s]: partition dim outermost so one DMA fills all k-tiles
        # of a column range with matching AP iteration order
        eT = np.ascontiguousarray(
            es.transpose(0, 2, 1).reshape(BPC, KT, 128, S).transpose(0, 2, 1, 3)
        ).astype(fp8)
        in_maps.append(
            {
                "encT": eT,
                "wh": wh_host,
                "we": we_host,
                "hiddenT": hT,
                "bvec": b_host,
                "vvec": v_host,
            }
        )
    return in_maps


def _run(in_maps, **kwargs):
    nc = _get_nc()
    try:
        return run_bass_kernel_spmd(
            nc, in_maps, core_ids=list(range(NCORES)), **kwargs
        )
    except Exception:
        # A first execution right after NEFF load has been seen to wedge the
        # device once; it recovers after a short pause. Retry once.
        import time as _time

        _time.sleep(20)
        return run_bass_kernel_spmd(
            nc, in_maps, core_ids=list(range(NCORES)), **kwargs
        )


def kernel(hidden, encoder_outputs, W, b, v):
    hidden = np.asarray(hidden, dtype=np.float32)
    encoder_outputs = np.asarray(encoder_outputs, dtype=np.float32)
    W = np.asarray(W, dtype=np.float32)
    b = np.asarray(b, dtype=np.float32)
    v = np.asarray(v, dtype=np.float32)

    in_maps = _make_in_maps(hidden, encoder_outputs, W, b, v)
    res = _run(in_maps)
    outs = [np.asarray(res.results[i]["out"], dtype=np.float32) for i in range(NCORES)]
    return np.concatenate(outs, axis=0).reshape(B, 1, S)


# revision 21
# speedup vs baseline: 1.1512x; 1.1465x over previous
# Bahdanau-attention kernel for TRN2, data-parallel over batch across 8 NeuronCores.
#
# reference math (B=16, S=2048, H=1024):
#   h_proj = hidden @ W[:, :H].T                      [B, H]
#   e_proj = einsum('bsh,gh->bsg', enc, W[:, H:])     [B, S, H]
#   scores = tanh(h_proj[:,None,:] + e_proj + b)      [B, S, H]
#   logits = scores @ v                               [B, S]
#   out    = softmax(logits, -1)[:, None, :]          [B, 1, S]
#
# Per-core layout (2 batches/core):
#   All matmuls contract over h (or g), so every SBUF operand is laid out with
#   the contraction dim on partitions. Host pre-transposes/pre-tiles:
#     encT[bb, p, k, s] = enc[2i+bb, s, 128k+p]       (fp8 e4m3)
#     we[p, j, k, m] = 32 * W[128j+m, 1024+128k+p]    (fp8 e4m3, W_e^T pre-scaled)
#     wh[p, j, k, m] = W[128j+m, 128k+p]              (bf16, W_h^T for h_proj)
#     hiddenT[p, k, bb] = hidden[2i+bb, 128k+p]       (bf16)
#     bvec[p, j] = b[128j+p] (f32);  vvec[p, j] = v[128j+p] (bf16)
#   On chip per (batch, s-block of 512):
#     for each g-tile j: psum[g,s] = sum_kp DoubleRow-MM(we pair, encT pair)
#       (fp8 DoubleRow: one MM contracts two k-tiles -> 4 MMs per group, ~2x rate)
#     scoresT = tanh(psum/32 + (h_projT + b)[g])   (ACT, per-partition bias, bf16 out)
#     logits_psum[1,s] += v_tile(j).T @ scoresT    (PE, bf16)
#     then per-block EXP from PSUM (no max-subtraction; logits bounded by ||v||_1),
#     accumulated partials, one reciprocal, scaled output row.

import numpy as np
import ml_dtypes

import concourse.bass as bass
import concourse.mybir as mybir
import concourse.tile as tile
from concourse import bacc
from concourse.bass_utils import run_bass_kernel_spmd
from concourse.tile_rust import add_dep_helper

B, S, H = 16, 2048, 1024
NCORES = 8
BPC = B // NCORES          # batches per core
KT = H // 128              # contraction tiles
GT = H // 128              # output (g) tiles
SBLK = 512                 # s-block (one PSUM bank of f32)
NSB = S // SBLK

BF16 = mybir.dt.bfloat16
F32 = mybir.dt.float32
FP8 = mybir.dt.float8e4
WSCALE = 32.0              # W_e pre-scaled into fp8's sweet range; undone in tanh's scale

_CACHE = {}


def _build():
    nc = bacc.Bacc("TRN2", target_bir_lowering=False, debug=False, num_devices=NCORES)

    encT_d = nc.dram_tensor("encT", [BPC, 128, KT, S], FP8, kind="ExternalInput")
    wh_d = nc.dram_tensor("wh", [128, GT, KT, 128], BF16, kind="ExternalInput")
    we_d = nc.dram_tensor("we", [128, GT, KT, 128], FP8, kind="ExternalInput")
    hiddenT_d = nc.dram_tensor("hiddenT", [128, KT, BPC], BF16, kind="ExternalInput")
    bvec_d = nc.dram_tensor("bvec", [128, GT], F32, kind="ExternalInput")
    vvec_d = nc.dram_tensor("vvec", [128, GT, 16], FP8, kind="ExternalInput")
    out_d = nc.dram_tensor("out", [BPC, S], F32, kind="ExternalOutput")

    ACT = mybir.ActivationFunctionType

    with tile.TileContext(nc) as tc:
        with (
            tc.tile_pool(name="const", bufs=1) as constp,
            tc.tile_pool(name="wp", bufs=1) as wp,
            tc.tile_pool(name="encp", bufs=1) as encp,
            tc.tile_pool(name="scp", bufs=2) as scp,
            tc.tile_pool(name="smallp", bufs=2) as smallp,
            tc.tile_pool(name="mps", bufs=3, space="PSUM") as mps,
            tc.tile_pool(name="lps", bufs=2, space="PSUM") as lps,
            tc.tile_pool(name="hps", bufs=2, space="PSUM") as hps,
        ):
            # --- tiny constants ---
            hiddenT_sb = constp.tile([128, KT, BPC], BF16, tag="hiddenT")
            nc.sync.dma_start(out=hiddenT_sb[:], in_=hiddenT_d[:])
            b_sb = constp.tile([128, GT], F32, tag="bvec")
            nc.sync.dma_start(out=b_sb[:], in_=bvec_d[:])
            v_sb = constp.tile([128, GT, 16], FP8, tag="vvec")
            nc.sync.dma_start(out=v_sb[:], in_=vvec_d[:])

            # --- weights: W_e^T in fp8 (DoubleRow main GEMM), W_h^T in bf16 ---
            we_sb = [None] * GT
            wh_sb = [None] * GT

            def load_we(j):
                t = wp.tile([128, KT, 128], FP8, name=f"we{j}", tag=f"we{j}")
                inst = nc.sync.dma_start(out=t[:], in_=we_d[:, j])
                we_sb[j] = t
                return inst

            def load_wh(j):
                t = wp.tile([128, KT, 128], BF16, name=f"wh{j}", tag=f"wh{j}")
                inst = nc.sync.dma_start(out=t[:], in_=wh_d[:, j])
                wh_sb[j] = t
                return inst

            # h_projT[j] = sum_k W_h(k,j).T @ hiddenT(k)  -> [128, BPC] per g-tile j,
            # then hb[:, j, bb] = h_projT + b (per-partition bias for the tanh).
            hb_sb = constp.tile([128, GT, BPC], F32, tag="hb")

            def hproj(j):
                hp = hps.tile([128, BPC], F32, tag="hp")
                for k in range(KT):
                    nc.tensor.matmul(
                        hp[:],
                        wh_sb[j][:, k, :],
                        hiddenT_sb[:, k, :],
                        start=(k == 0),
                        stop=(k == KT - 1),
                    )
                nc.vector.tensor_scalar_add(hb_sb[:, j, :], hp[:], b_sb[:, j : j + 1])

            # DMA order tuned so the first main matmul group can start early:
            # We(j=0), Wh(j=0), enc b0 (first s-half), then alternating We/Wh columns.
            load_we(0)
            load_wh(0)
            hproj(0)

            enc_sb = [
                encp.tile([128, KT, S], FP8, name=f"enc{bb}", tag=f"enc{bb}")
                for bb in range(BPC)
            ]

            def load_enc(bb, lo, size):
                sl = slice(lo, lo + size)
                return nc.sync.dma_start(
                    out=enc_sb[bb][:, :, sl], in_=encT_d[bb][:, :, sl]
                )

            # DMA waves: what the first matmul group needs streams immediately;
            # later waves are gated on compute milestones (add_dep_helper below)
            # so the critical wave gets the full HBM bandwidth.
            dma_waves = [[], [], []]
            load_enc(0, 0, SBLK)
            for j in range(1, GT):
                dma_waves[0].append(load_we(j))
                dma_waves[0].append(load_wh(j))
                hproj(j)
            dma_waves[0].append(load_enc(0, SBLK, SBLK))
            dma_waves[1].append(load_enc(0, 2 * SBLK, 2 * SBLK))
            dma_waves[1].append(load_enc(1, 0, 2 * SBLK))
            dma_waves[2].append(load_enc(1, 2 * SBLK, 2 * SBLK))

            # --- main loop ---
            # Softmax note: no max-subtraction — |logits| <= ||v||_1 * max|tanh| ~ 9.6,
            # so exp() cannot overflow in f32 and the softmax ratio is unchanged.
            gates = {}
            mm_gate = None
            for bb in range(BPC):
                exps = smallp.tile([1, S], F32, name=f"exps{bb}", tag=f"exps{bb}")
                parts = smallp.tile([1, NSB], F32, name=f"parts{bb}", tag=f"parts{bb}")
                for sb in range(NSB):
                    sl = slice(sb * SBLK, (sb + 1) * SBLK)
                    scps = []
                    for jp in range(GT // 2):
                        mp2 = [None, None]
                        for half in range(2):
                            j = 2 * jp + half
                            mp2[half] = mps.tile(
                                [128, SBLK], F32, tag="mp", name=f"mp{half}"
                            )
                            for kp in range(KT // 2):
                                mm = nc.tensor.matmul(
                                    mp2[half][:],
                                    we_sb[j][:, 2 * kp : 2 * kp + 2, :],
                                    enc_sb[bb][:, 2 * kp : 2 * kp + 2, sl],
                                    start=(kp == 0),
                                    stop=(kp == KT // 2 - 1),
                                    perf_mode=mybir.MatmulPerfMode.DoubleRow,
                                )
                                if mm_gate is None:
                                    mm_gate = mm
                        # fp8 scores, stored as a g-tile pair for the DoubleRow v-dot
                        sc2 = scp.tile(
                            [128, 2, SBLK], FP8, name=f"sc{jp}", tag=f"sc{jp}"
                        )
                        for half in range(2):
                            j = 2 * jp + half
                            act_inst = nc.scalar.activation(
                                sc2[:, half, :], mp2[half][:], ACT.Tanh,
                                bias=hb_sb[:, j, bb : bb + 1],
                                scale=1.0 / WSCALE,
                            )
                            gates[(bb, sb, j)] = act_inst
                        scps.append(sc2)
                    # v-dot: DoubleRow over g-tile pairs; v is x16 in fp8 lane 0,
                    # so logits land in psum row 0 scaled by 16 (undone in EXP)
                    lp = lps.tile([16, SBLK], F32, tag="lp")
                    for jp in range(GT // 2):
                        nc.tensor.matmul(
                            lp[:],
                            v_sb[:, 2 * jp : 2 * jp + 2, :],
                            scps[jp][:],
                            start=(jp == 0),
                            stop=(jp == GT // 2 - 1),
                            perf_mode=mybir.MatmulPerfMode.DoubleRow,
                        )
                    nc.scalar.activation(
                        exps[:, sl], lp[0:1, :], ACT.Exp,
                        accum_out=parts[:, sb : sb + 1],
                        scale=1.0 / 16.0,
                    )

                ssum = smallp.tile([1, 1], F32, tag="ssum")
                nc.vector.tensor_reduce(
                    ssum[:], parts[:], axis=mybir.AxisListType.X,
                    op=mybir.AluOpType.add,
                )
                rsum = smallp.tile([1, 1], F32, tag="rsum")
                nc.vector.reciprocal(rsum[:], ssum[:])
                outrow = smallp.tile([1, S], F32, name=f"outrow{bb}", tag=f"outrow{bb}")
                nc.vector.tensor_scalar_mul(outrow[:], exps[:], rsum[:])
                nc.sync.dma_start(out=out_d[bb : bb + 1, :], in_=outrow[:])

            # gate the later DMA waves on compute progress: wave 1 after the
            # first tanh of (b0, sb0); wave 2 after (b0, sb1) finishes.
            for inst in dma_waves[0]:
                add_dep_helper(
                    inst.ins, mm_gate.ins, sync=True,
                    reason="dma wave 0 gated on first main matmul",
                )
            for inst in dma_waves[1]:
                add_dep_helper(
                    inst.ins, gates[(0, 0, 0)].ins, sync=True,
                    reason="dma wave 1 gated on first tanh",
                )
            for inst in dma_waves[2]:
                add_dep_helper(
                    inst.ins, gates[(0, 1, 7)].ins, sync=True,
                    reason="dma wave 2 gated on (b0,sb1) tanh",
                )

    nc.compile()
    return nc


def _get_nc():
    if "nc" not in _CACHE:
        _CACHE["nc"] = _build()
    return _CACHE["nc"]


def _make_in_maps(hidden, encoder_outputs, W, b, v):
    bf = ml_dtypes.bfloat16
    fp8 = ml_dtypes.float8_e4m3
    WT = np.ascontiguousarray(W.T)  # [2H, H]; WT[hin, gout]
    w_tiles = WT.reshape(2, KT, 128, GT, 128).transpose(0, 2, 3, 1, 4)  # [half, p, j, k, m]
    wh_host = np.ascontiguousarray(w_tiles[0]).astype(bf)
    we_host = np.ascontiguousarray(w_tiles[1] * WSCALE).astype(fp8)
    b_host = np.ascontiguousarray(b.reshape(GT, 128).T).astype(np.float32)
    v_host = np.zeros((128, GT, 16), dtype=fp8)
    v_host[:, :, 0] = (v.reshape(GT, 128).T * 16.0).astype(fp8)

    in_maps = []
    for i in range(NCORES):
        hs = hidden[BPC * i : BPC * (i + 1)]  # [BPC, H]
        es = encoder_outputs[BPC * i : BPC * (i + 1)]  # [BPC, S, H]
        hT = np.ascontiguousarray(
            hs.T.reshape(KT, 128, BPC).transpose(1, 0, 2)
        ).astype(bf)
        # [bb, p, k, s]: partition dim outermost so one DMA fills all k-tiles
        # of a column range with matching AP iteration order
        eT = np.ascontiguousarray(
            es.transpose(0, 2, 1).reshape(BPC, KT, 128, S).transpose(0, 2, 1, 3)
        ).astype(fp8)
        in_maps.append(
            {
                "encT": eT,
                "wh": wh_host,
                "we": we_host,
                "hiddenT": hT,
                "bvec": b_host,
                "vvec": v_host,
            }
        )
    return in_maps


def _run(in_maps, **kwargs):
    nc = _get_nc()
    try:
        return run_bass_kernel_spmd(
            nc, in_maps, core_ids=list(range(NCORES)), **kwargs
        )
    except Exception:
        # A first execution right after NEFF load has been seen to wedge the
        # device once; it recovers after a short pause. Retry once.
        import time as _time

        _time.sleep(20)
        return run_bass_kernel_spmd(
            nc, in_maps, core_ids=list(range(NCORES)), **kwargs
        )


def kernel(hidden, encoder_outputs, W, b, v):
    hidden = np.asarray(hidden, dtype=np.float32)
    encoder_outputs = np.asarray(encoder_outputs, dtype=np.float32)
    W = np.asarray(W, dtype=np.float32)
    b = np.asarray(b, dtype=np.float32)
    v = np.asarray(v, dtype=np.float32)

    in_maps = _make_in_maps(hidden, encoder_outputs, W, b, v)
    res = _run(in_maps)
    outs = [np.asarray(res.results[i]["out"], dtype=np.float32) for i in range(NCORES)]
    return np.concatenate(outs, axis=0).reshape(B, 1, S)


# revision 22
# speedup vs baseline: 1.2066x; 1.0482x over previous
# Bahdanau-attention kernel for TRN2, data-parallel over batch across 8 NeuronCores.
#
# reference math (B=16, S=2048, H=1024):
#   h_proj = hidden @ W[:, :H].T                      [B, H]
#   e_proj = einsum('bsh,gh->bsg', enc, W[:, H:])     [B, S, H]
#   scores = tanh(h_proj[:,None,:] + e_proj + b)      [B, S, H]
#   logits = scores @ v                               [B, S]
#   out    = softmax(logits, -1)[:, None, :]          [B, 1, S]
#
# Per-core layout (2 batches/core):
#   All matmuls contract over h (or g), so every SBUF operand is laid out with
#   the contraction dim on partitions. Host pre-transposes/pre-tiles:
#     encT[bb, p, k, s] = enc[2i+bb, s, 128k+p]       (fp8 e4m3)
#     we[p, j, k, m] = 32 * W[128j+m, 1024+128k+p]    (fp8 e4m3, W_e^T pre-scaled)
#     wh[p, j, k, m] = W[128j+m, 128k+p]              (bf16, W_h^T for h_proj)
#     hiddenT[p, k, bb] = hidden[2i+bb, 128k+p]       (bf16)
#     bvec[p, j] = b[128j+p] (f32);  vvec[p, j, 0] = 16*v[128j+p] (fp8, lane 0 of 16)
#   On chip per (batch, s-block of 512):
#     for each g-tile j: psum[g,s] = sum_kp DoubleRow-MM(we pair, encT pair)
#       (fp8 DoubleRow: one MM contracts two k-tiles -> 4 MMs per group, ~2x rate)
#     scoresT = tanh(psum/32 + (h_projT + b)[g])   (ACT, per-partition bias, fp8 out)
#     logits_psum += DoubleRow-MM(v pair, scoresT pair)  (PE, fp8; v x16 in lane 0)
#     then per-block EXP from PSUM (no max-subtraction; logits bounded by ||v||_1),
#     accumulated partials, one reciprocal, scaled output row.

import numpy as np
import ml_dtypes

import concourse.bass as bass
import concourse.mybir as mybir
import concourse.tile as tile
from concourse import bacc
from concourse.bass_utils import run_bass_kernel_spmd
from concourse.tile_rust import add_dep_helper

B, S, H = 16, 2048, 1024
NCORES = 8
BPC = B // NCORES          # batches per core
KT = H // 128              # contraction tiles
GT = H // 128              # output (g) tiles
SBLK = 512                 # s-block (one PSUM bank of f32)
NSB = S // SBLK

BF16 = mybir.dt.bfloat16
F32 = mybir.dt.float32
FP8 = mybir.dt.float8e4
WSCALE = 32.0              # W_e pre-scaled into fp8's sweet range; undone in tanh's scale

_CACHE = {}


def _build():
    nc = bacc.Bacc("TRN2", target_bir_lowering=False, debug=False, num_devices=NCORES)

    encT_d = nc.dram_tensor("encT", [BPC, 128, KT, S], FP8, kind="ExternalInput")
    wh_d = nc.dram_tensor("wh", [128, GT, KT, 128], BF16, kind="ExternalInput")
    we_d = nc.dram_tensor("we", [128, GT, KT, 128], FP8, kind="ExternalInput")
    hiddenT_d = nc.dram_tensor("hiddenT", [128, KT, BPC], BF16, kind="ExternalInput")
    bvec_d = nc.dram_tensor("bvec", [128, GT], F32, kind="ExternalInput")
    vvec_d = nc.dram_tensor("vvec", [128, GT, 16], FP8, kind="ExternalInput")
    out_d = nc.dram_tensor("out", [BPC, S], F32, kind="ExternalOutput")

    ACT = mybir.ActivationFunctionType

    with tile.TileContext(nc) as tc:
        with (
            tc.tile_pool(name="const", bufs=1) as constp,
            tc.tile_pool(name="wp", bufs=1) as wp,
            tc.tile_pool(name="encp", bufs=1) as encp,
            tc.tile_pool(name="scp", bufs=2) as scp,
            tc.tile_pool(name="smallp", bufs=2) as smallp,
            tc.tile_pool(name="mps", bufs=3, space="PSUM") as mps,
            tc.tile_pool(name="lps", bufs=2, space="PSUM") as lps,
            tc.tile_pool(name="hps", bufs=2, space="PSUM") as hps,
        ):
            # --- tiny constants ---
            hiddenT_sb = constp.tile([128, KT, BPC], BF16, tag="hiddenT")
            nc.sync.dma_start(out=hiddenT_sb[:], in_=hiddenT_d[:])
            b_sb = constp.tile([128, GT], F32, tag="bvec")
            nc.sync.dma_start(out=b_sb[:], in_=bvec_d[:])
            v_sb = constp.tile([128, GT, 16], FP8, tag="vvec")
            nc.sync.dma_start(out=v_sb[:], in_=vvec_d[:])

            # --- weights: W_e^T in fp8 (DoubleRow main GEMM), W_h^T in bf16 ---
            we_sb = [None] * GT
            wh_sb = [None] * GT

            def load_we(j):
                t = wp.tile([128, KT, 128], FP8, name=f"we{j}", tag=f"we{j}")
                inst = nc.sync.dma_start(out=t[:], in_=we_d[:, j])
                we_sb[j] = t
                return inst

            def load_wh(j):
                t = wp.tile([128, KT, 128], BF16, name=f"wh{j}", tag=f"wh{j}")
                inst = nc.sync.dma_start(out=t[:], in_=wh_d[:, j])
                wh_sb[j] = t
                return inst

            # h_projT[j] = sum_k W_h(k,j).T @ hiddenT(k)  -> [128, BPC] per g-tile j,
            # then hb[:, j, bb] = h_projT + b (per-partition bias for the tanh).
            hb_sb = constp.tile([128, GT, BPC], F32, tag="hb")

            def hproj(j):
                hp = hps.tile([128, BPC], F32, tag="hp")
                for k in range(KT):
                    nc.tensor.matmul(
                        hp[:],
                        wh_sb[j][:, k, :],
                        hiddenT_sb[:, k, :],
                        start=(k == 0),
                        stop=(k == KT - 1),
                    )
                nc.vector.tensor_scalar_add(hb_sb[:, j, :], hp[:], b_sb[:, j : j + 1])

            # DMA order tuned so the first main matmul group can start early:
            # We(j=0), Wh(j=0), enc b0 (first s-half), then alternating We/Wh columns.
            load_we(0)
            load_wh(0)
            hproj(0)

            enc_sb = [
                encp.tile([128, KT, S], FP8, name=f"enc{bb}", tag=f"enc{bb}")
                for bb in range(BPC)
            ]

            def load_enc(bb, lo, size):
                sl = slice(lo, lo + size)
                return nc.sync.dma_start(
                    out=enc_sb[bb][:, :, sl], in_=encT_d[bb][:, :, sl]
                )

            # DMA waves: what the first matmul group needs streams immediately;
            # later waves are gated on compute milestones (add_dep_helper below)
            # so the critical wave gets the full HBM bandwidth.
            dma_waves = [[], [], []]
            load_enc(0, 0, SBLK)
            for j in range(1, GT):
                dma_waves[0].append(load_we(j))
                dma_waves[0].append(load_wh(j))
                hproj(j)
            dma_waves[0].append(load_enc(0, SBLK, SBLK))
            dma_waves[1].append(load_enc(0, 2 * SBLK, 2 * SBLK))
            dma_waves[1].append(load_enc(1, 0, 2 * SBLK))
            dma_waves[2].append(load_enc(1, 2 * SBLK, 2 * SBLK))

            # --- main loop ---
            # Softmax note: no max-subtraction — |logits| <= ||v||_1 * max|tanh| ~ 9.6,
            # so exp() cannot overflow in f32 and the softmax ratio is unchanged.
            gates = {}
            mm_gate = None
            for bb in range(BPC):
                exps = smallp.tile([1, S], F32, name=f"exps{bb}", tag=f"exps{bb}")
                parts = smallp.tile([1, NSB], F32, name=f"parts{bb}", tag=f"parts{bb}")
                for sb in range(NSB):
                    sl = slice(sb * SBLK, (sb + 1) * SBLK)
                    scps = []
                    for jp in range(GT // 2):
                        mp2 = [None, None]
                        for half in range(2):
                            j = 2 * jp + half
                            mp2[half] = mps.tile(
                                [128, SBLK], F32, tag="mp", name=f"mp{half}"
                            )
                            for kp in range(KT // 2):
                                mm = nc.tensor.matmul(
                                    mp2[half][:],
                                    we_sb[j][:, 2 * kp : 2 * kp + 2, :],
                                    enc_sb[bb][:, 2 * kp : 2 * kp + 2, sl],
                                    start=(kp == 0),
                                    stop=(kp == KT // 2 - 1),
                                    perf_mode=mybir.MatmulPerfMode.DoubleRow,
                                )
                                if mm_gate is None:
                                    mm_gate = mm
                        # fp8 scores, stored as a g-tile pair for the DoubleRow v-dot
                        sc2 = scp.tile(
                            [128, 2, SBLK], FP8, name=f"sc{jp}", tag=f"sc{jp}"
                        )
                        for half in range(2):
                            j = 2 * jp + half
                            act_inst = nc.scalar.activation(
                                sc2[:, half, :], mp2[half][:], ACT.Tanh,
                                bias=hb_sb[:, j, bb : bb + 1],
                                scale=1.0 / WSCALE,
                            )
                            gates[(bb, sb, j)] = act_inst
                        scps.append(sc2)
                    # v-dot: DoubleRow over g-tile pairs; v is x16 in fp8 lane 0,
                    # so logits land in psum row 0 scaled by 16 (undone in EXP)
                    lp = lps.tile([16, SBLK], F32, tag="lp")
                    for jp in range(GT // 2):
                        nc.tensor.matmul(
                            lp[:],
                            v_sb[:, 2 * jp : 2 * jp + 2, :],
                            scps[jp][:],
                            start=(jp == 0),
                            stop=(jp == GT // 2 - 1),
                            perf_mode=mybir.MatmulPerfMode.DoubleRow,
                        )
                    nc.scalar.activation(
                        exps[:, sl], lp[0:1, :], ACT.Exp,
                        accum_out=parts[:, sb : sb + 1],
                        scale=1.0 / 16.0,
                    )

                ssum = smallp.tile([1, 1], F32, tag="ssum")
                nc.vector.tensor_reduce(
                    ssum[:], parts[:], axis=mybir.AxisListType.X,
                    op=mybir.AluOpType.add,
                )
                rsum = smallp.tile([1, 1], F32, tag="rsum")
                nc.vector.reciprocal(rsum[:], ssum[:])
                outrow = smallp.tile([1, S], F32, name=f"outrow{bb}", tag=f"outrow{bb}")
                nc.vector.tensor_scalar_mul(outrow[:], exps[:], rsum[:])
                nc.sync.dma_start(out=out_d[bb : bb + 1, :], in_=outrow[:])

            # gate the later DMA waves on compute progress: wave 1 after the
            # first tanh of (b0, sb0); wave 2 after (b0, sb1) finishes.
            for inst in dma_waves[0]:
                add_dep_helper(
                    inst.ins, mm_gate.ins, sync=True,
                    reason="dma wave 0 gated on first main matmul",
                )
            for inst in dma_waves[1]:
                add_dep_helper(
                    inst.ins, gates[(0, 0, 0)].ins, sync=True,
                    reason="dma wave 1 gated on first tanh",
                )
            for inst in dma_waves[2]:
                add_dep_helper(
                    inst.ins, gates[(0, 1, 7)].ins, sync=True,
                    reason="dma wave 2 gated on (b0,sb1) tanh",
                )

    nc.compile()
    return nc


def _get_nc():
    if "nc" not in _CACHE:
        _CACHE["nc"] = _build()
    return _CACHE["nc"]


def _make_in_maps(hidden, encoder_outputs, W, b, v):
    bf = ml_dtypes.bfloat16
    fp8 = ml_dtypes.float8_e4m3
    WT = np.ascontiguousarray(W.T)  # [2H, H]; WT[hin, gout]
    w_tiles = WT.reshape(2, KT, 128, GT, 128).transpose(0, 2, 3, 1, 4)  # [half, p, j, k, m]
    wh_host = np.ascontiguousarray(w_tiles[0]).astype(bf)
    we_host = np.ascontiguousarray(w_tiles[1] * WSCALE).astype(fp8)
    b_host = np.ascontiguousarray(b.reshape(GT, 128).T).astype(np.float32)
    v_host = np.zeros((128, GT, 16), dtype=fp8)
    v_host[:, :, 0] = (v.reshape(GT, 128).T * 16.0).astype(fp8)

    in_maps = []
    for i in range(NCORES):
        hs = hidden[BPC * i : BPC * (i + 1)]  # [BPC, H]
        es = encoder_outputs[BPC * i : BPC * (i + 1)]  # [BPC, S, H]
        hT = np.ascontiguousarray(
            hs.T.reshape(KT, 128, BPC).transpose(1, 0, 2)
        ).astype(bf)
        # [bb, p, k, s]: partition dim outermost so one DMA fills all k-tiles
        # of a column range with matching AP iteration order
        eT = np.ascontiguousarray(
            es.transpose(0, 2, 1).reshape(BPC, KT, 128, S).transpose(0, 2, 1, 3)
        ).astype(fp8)
        in_maps.append(
            {
                "encT": eT,
                "wh": wh_host,
                "we": we_host,
                "hiddenT": hT,
                "bvec": b_host,
                "vvec": v_host,
            }
        )
    return in_maps


def _run(in_maps, **kwargs):
    nc = _get_nc()
    try:
        return run_bass_kernel_spmd(
            nc, in_maps, core_ids=list(range(NCORES)), **kwargs
        )
    except Exception:
        # A first execution right after NEFF load has been seen to wedge the
        # device once; it recovers after a short pause. Retry once.
        import time as _time

        _time.sleep(20)
        return run_bass_kernel_spmd(
            nc, in_maps, core_ids=list(range(NCORES)), **kwargs
        )


def kernel(hidden, encoder_outputs, W, b, v):
    hidden = np.asarray(hidden, dtype=np.float32)
    encoder_outputs = np.asarray(encoder_outputs, dtype=np.float32)
    W = np.asarray(W, dtype=np.float32)
    b = np.asarray(b, dtype=np.float32)
    v = np.asarray(v, dtype=np.float32)

    in_maps = _make_in_maps(hidden, encoder_outputs, W, b, v)
    res = _run(in_maps)
    outs = [np.asarray(res.results[i]["out"], dtype=np.float32) for i in range(NCORES)]
    return np.concatenate(outs, axis=0).reshape(B, 1, S)
